# revision 37
# baseline (speedup 1.0000x reference)
"""Llama GQA attention (B=1, Q=1024, PAST=3072, HID=4096, NH=32, NKV=8, HD=128)
tensor-parallel over heads across 8 NeuronCores.

Per core c: kv head c, query heads 4c..4c+3. Each core computes its partial
o_proj contribution [1024, 4096] in bf16; the host sums the 8 partials in f32.

v3 layout (vs v2): restructured so the ScalarE exp stream (the stage-2
bottleneck, ~35us/head) starts ~50us earlier and the PE never starves:
  - pass 1 computes ONLY q head 0 + k + v (6 N=512 MMs per hid k-tile,
    DMA-paced); head 0's attention starts right after the k/q0 ropes.
  - q proj for head h+1 runs as PE *filler* inside head h's attention loop
    (the attention loop is exp-bound on ScalarE, leaving PE idle slots).
  - v proj is W-stationary (out [d,s], 32 N=512 MMs per group instead of
    128 N=128 hs-stationary MMs); [s,d] layout for attention is recovered
    with 8 SBUF->SBUF xbar DMA transposes (off-engine).
  - scores in f32 PSUM [128,1024] supertiles (1 kv tile x full q), exp is
    one ACTIVATE per supertile; diagonal masking via 0/1 mask multiplies
    split GpSimd/DVE (as v2).
  - softmax tail: denominator partition-reduce via ones-matmuls into a
    [2,512] PSUM tile, reciprocal_approx_fast (0.9us vs 3.3us iterative
    divide), broadcast matmul result consumed directly from PSUM by the
    normalize multiply (no bc copy). Tail is deferred into the next head.
  - o_proj: [128,512] chunks, h-accumulated in PSUM; the first chunks' h0/h1
    matmuls run as head-3 filler; exp table preloaded at t=0.
"""

import math
import os as _os
import numpy as np
import ml_dtypes
from contextlib import ExitStack

import bass_rust
import concourse.bass as bass
import concourse.mybir as mybir
import concourse.tile as tile
from concourse.vector_clock import ScopedClock
from concourse.bass_utils import run_bass_kernel_spmd

# ---------------------------------------------------------------------------
# Workaround: walrus in this image rejects >1 sem wait on CTRL-class
# instructions (Drain/NoOp). TileContext's tail drain waits on every touched
# logical processor. Split the waits across preceding sync-engine nops.
MAX_WAITS = 1


def _split_waits(nc, inst):
    si = inst.ins.sync_info
    if si is None:
        return
    waits = list(si.on_wait)
    if len(waits) <= MAX_WAITS:
        return
    inst.ins.sync_info = bass_rust.SyncInfo(
        on_wait=waits[:MAX_WAITS], on_update=list(si.on_update)
    )
    rest = waits[MAX_WAITS:]
    while rest:
        extra = nc.sync.nop(nofuse=True)
        extra.ins.sync_info = bass_rust.SyncInfo(on_wait=rest[:MAX_WAITS], on_update=[])
        rest = rest[MAX_WAITS:]


def _drain_and_barrier_split(self, tick_clock, wait_clock):
    nc = self.nc
    carrier = nc.sync.nop(nofuse=True)
    wait_clock.add_sem_waits(carrier.ins, ScopedClock({None: tick_clock.global_clock}))
    _split_waits(nc, carrier)
    nc.sync.drain()
    nc.all_engine_barrier()
    popped = nc._tile_sem_poison_stack.pop()
    assert popped is self._sem_poison
    nc.clear_and_free_semaphores(list(self.sems.allocated().values()))
    nc.all_engine_barrier()


tile.TileContext._drain_and_barrier = _drain_and_barrier_split
# ---------------------------------------------------------------------------

# ---------------------------------------------------------------------------
# General wait-cap legalization: this walrus rejects instructions carrying
# more than a couple of sem waits. Post-process the BIR JSON: hoist overflow
# waits onto engine-matched NoOps inserted immediately before the offender
# (same engine queue -> same ordering semantics).
import json as _json

_CTRL_OPS = {"NoOp", "Drain", "EventSemaphore"}
_CAP_CTRL = 1
_CAP_OTHER = 1
if not hasattr(bass.Bass, "_orig_to_json_bytes"):
    bass.Bass._orig_to_json_bytes = bass.Bass.to_json_bytes
_orig_to_json_bytes = bass.Bass._orig_to_json_bytes


def _legalized_to_json_bytes(self, *a, **k):
    raw = _orig_to_json_bytes(self, *a, **k)
    m = _json.loads(raw)
    ctr = [0]
    changed = False
    for fn in m.get("functions", []):
        for blk in fn.get("blocks", []):
            insts = blk.get("instructions", [])
            out = []
            for ins in insts:
                si = ins.get("sync_info")
                if si:
                    waits = si.get("on_wait") or []
                    cap = _CAP_CTRL if ins.get("opcode") in _CTRL_OPS else _CAP_OTHER
                    if len(waits) > cap:
                        changed = True
                        rest = waits[:-cap]
                        si["on_wait"] = waits[-cap:]
                        while rest:
                            ctr[0] += 1
                            out.append({
                                "debug": ins.get("debug", 0),
                                "engine": ins["engine"],
                                "ins": [], "outs": [],
                                "name": f"{ins['name']}_lw{ctr[0]}",
                                "opcode": "NoOp",
                                "sync_info": {"on_wait": rest[:_CAP_CTRL],
                                              "on_update": []},
                            })
                            rest = rest[_CAP_CTRL:]
                out.append(ins)
            blk["instructions"] = out
    if not changed:
        return raw
    return _json.dumps(m).encode()


bass.Bass.to_json_bytes = _legalized_to_json_bytes
# ---------------------------------------------------------------------------


B, Q, PAST, HID = 1, 1024, 3072, 4096
NH, NKV, HD = 32, 8, 128
KV = PAST + Q           # 4096
NCORES = 8
HPC = NH // NCORES      # 4 query heads per core
ROPE_THETA = 10000.0
EXP_SHIFT = -20.0       # constant softmax shift (cancels exactly per row)

F32 = mybir.dt.float32
BF16 = mybir.dt.bfloat16

N_KT = KV // 128        # 32 kv tiles
N_HK = HID // 128       # 32 hid k-tiles
GRP = 512               # query group width
N_G = Q // GRP          # 2 groups
N_PV = PAST // 128      # 24 past-v tiles

# attention supertile order: 1 kv tile x full q (narrow tiles 28-31 only vs
# q 512:1024). Past tiles first so the attn-start flags never wait on new kv;
# diagonal/narrow (masked) tiles interleaved 1:1 to spread the mask muls.
ORDER = [0, 24, 1, 25, 2, 26, 3, 27, 4, 28, 5, 29, 6, 30, 7, 31] + list(range(8, 24))
ATTN_FIRST, ATTN_LAST = 0, 23

LAST_RESULTS = None     # test harness reads exec_time_ns from here


def _build_program():
    nc = bass.Bass()
    hst = nc.declare_dram_parameter("hst", [128, N_HK, Q], BF16, isOutput=False)
    wqt = nc.declare_dram_parameter("wqt", [128, HPC, N_HK, 128], BF16, isOutput=False)
    wkvt = nc.declare_dram_parameter("wkvt", [128, N_HK, 256], BF16, isOutput=False)
    pastkt = nc.declare_dram_parameter("pastkt", [128, PAST], BF16, isOutput=False)
    pastv = nc.declare_dram_parameter("pastv", [128, PAST], BF16, isOutput=False)
    maskt = nc.declare_dram_parameter("maskt", [128, 2048], BF16, isOutput=False)
    trit = nc.declare_dram_parameter("trit", [128, 128], BF16, isOutput=False)
    # rope tables in [d, seq] layout; q tables pre-scaled by 1/sqrt(HD)
    cosq = nc.declare_dram_parameter("cosq", [128, Q], BF16, isOutput=False)
    sinq = nc.declare_dram_parameter("sinq", [128, Q], BF16, isOutput=False)
    cosk = nc.declare_dram_parameter("cosk", [128, Q], BF16, isOutput=False)
    sink = nc.declare_dram_parameter("sink", [128, Q], BF16, isOutput=False)
    prot = nc.declare_dram_parameter("prot", [128, 128], BF16, isOutput=False)
    sel2 = nc.declare_dram_parameter("sel2", [2, 256], BF16, isOutput=False)
    wot = nc.declare_dram_parameter("wot", [128, HPC * HID], BF16, isOutput=False)
    outp = nc.declare_dram_parameter("outp", [Q, HID], BF16, isOutput=True)

    with TileCtx(nc) as (tc, st):
        cpool = st.enter_context(tc.tile_pool(name="const", bufs=1))
        kvpool = st.enter_context(tc.tile_pool(name="kvres", bufs=1))
        qtpool = st.enter_context(tc.tile_pool(name="qt", bufs=1))
        apool = st.enter_context(tc.tile_pool(name="attn", bufs=1))
        hspool = st.enter_context(tc.tile_pool(name="hsw", bufs=1))
        wopool = st.enter_context(tc.tile_pool(name="wo", bufs=1))

        # ---- consts / residents ----
        ones2a = cpool.tile([128, 2], BF16)
        nc.vector.memset(ones2a[:], 0.0)
        nc.vector.memset(ones2a[:, 0:1], 1.0)
        ones2b = cpool.tile([128, 2], BF16)
        nc.vector.memset(ones2b[:], 0.0)
        nc.vector.memset(ones2b[:, 1:2], 1.0)
        sel2_sb = cpool.tile([2, 256], BF16)
        sel_a = sel2_sb[:, 0:128]
        sel_b = sel2_sb[:, 128:256]
        shift_sb = cpool.tile([128, 1], F32)
        nc.vector.memset(shift_sb[:], EXP_SHIFT)
        prot_sb = cpool.tile([128, 128], BF16)
        mask_sb = cpool.tile([128, 2048], BF16)
        tri_sb = cpool.tile([128, 128], BF16)

        # exp table preload: hide the ~2.7us ACT_TABLE_LOAD under startup DMA
        warm_i = cpool.tile([128, 1], F32)
        nc.vector.memset(warm_i[:], 0.0)
        warm_o = cpool.tile([128, 1], BF16)
        nc.scalar.activation(warm_o[:], warm_i[:],
                             mybir.ActivationFunctionType.Exp,
                             bias=shift_sb[:], scale=1.0)

        kt_sb = kvpool.tile([128, KV], BF16)
        v_sb = kvpool.tile([128, N_KT * 128], BF16)
        qt_sb = [qtpool.tile([128, Q], BF16, tag=f"qt{h}", name=f"qt{h}")
                 for h in range(HPC)]
        at_sb = [apool.tile([128, Q], BF16, tag=f"at{h}", name=f"at{h}")
                 for h in range(HPC)]

        hs_sb = hspool.tile([128, N_HK, Q], BF16)
        wq_sb = hspool.tile([128, HPC, N_HK, 128], BF16)
        cosq_sb = hspool.tile([128, Q], BF16)
        sinq_sb = hspool.tile([128, Q], BF16)
        wo_sb = wopool.tile([128, HPC * HID], BF16)
        # shared aux PSUM (rope rot + softmax tail, temporally disjoint):
        # one [128,512]-f32 bank ring; opened first, closed last
        rot_stack = ExitStack()
        auxps = rot_stack.enter_context(
            tc.tile_pool(name="auxps", bufs=1, space="PSUM"))
        rpool = rot_stack.enter_context(tc.tile_pool(name="rope", bufs=2))
        q2_stack = ExitStack()
        q2ps = q2_stack.enter_context(
            tc.tile_pool(name="q2ps", bufs=1, space="PSUM"))
        # pass-1-only residents (freed before the stage-2 pools allocate)
        p1res_stack = ExitStack()
        p1res = p1res_stack.enter_context(tc.tile_pool(name="p1res", bufs=1))
        wkv_sb = p1res.tile([128, N_HK, 256], BF16)
        cosk_sb = p1res.tile([128, Q], BF16)
        sink_sb = p1res.tile([128, Q], BF16)

        # ---- DMA issue order = consumption order ----
        # pass-1-critical: hs + wq(head0) + wkv, finely chunked up front
        bounds = [0, 1, 2, 3, 4, 5, 6, 8, 10, 12, 14, 16, 20, 24, 28, 32]
        for i in range(len(bounds) - 1):
            s, e = bounds[i], bounds[i + 1]
            nc.sync.dma_start(hs_sb[:, s:e, :], hst[:, s:e, :])
            nc.sync.dma_start(wq_sb[:, 0, s:e, :], wqt[:, 0, s:e, :])
            nc.sync.dma_start(wkv_sb[:, s:e, :], wkvt[:, s:e, :])
        # rope tables + consts (needed ~50us)
        nc.sync.dma_start(cosq_sb[:], cosq[:])
        nc.sync.dma_start(sinq_sb[:], sinq[:])
        nc.sync.dma_start(cosk_sb[:], cosk[:])
        nc.sync.dma_start(sink_sb[:], sink[:])
        nc.sync.dma_start(prot_sb[:], prot[:])
        nc.sync.dma_start(mask_sb[:], maskt[:])
        nc.sync.dma_start(tri_sb[:], trit[:])
        nc.sync.dma_start(sel2_sb[:], sel2[:])
        # past kv (needed at head-0 attention ~55us)
        nc.sync.dma_start(kt_sb[:, :PAST], pastkt[:])
        nc.sync.dma_start(v_sb[:, : N_PV * 128], pastv[:])
        # filler q weights (heads 1-3, needed from ~55us)
        for h in range(1, HPC):
            nc.sync.dma_start(wq_sb[:, h, :, :], wqt[:, h, :, :])
        # wo last (needed ~220us)
        for h in range(HPC):
            nc.sync.dma_start(
                wo_sb[:, h * HID:(h + 1) * HID], wot[:, h * HID:(h + 1) * HID]
            )

        def rope(dst_bf, src_ps, cos_t, sin_t, g):
            """dst_bf [128 d, 512 s] <- RoPE applied in [d, s] layout."""
            c = cos_t[:, g * GRP:(g + 1) * GRP]
            s = sin_t[:, g * GRP:(g + 1) * GRP]
            q_f = rpool.tile([128, GRP], BF16, tag="qf", name="q_f")
            nc.scalar.copy(q_f[:], src_ps[:])
            rot_ps = auxps.tile([128, GRP], F32, tag="aux", name="rot_ps")
            nc.tensor.matmul(rot_ps[:], prot_sb[:], q_f[:], start=True, stop=True)
            t1 = rpool.tile([128, GRP], F32, tag="t1", name="t1")
            nc.vector.tensor_mul(t1[:], src_ps[:], c)
            t2 = rpool.tile([128, GRP], F32, tag="t2", name="t2")
            nc.vector.tensor_mul(t2[:], rot_ps[:], s)
            nc.vector.tensor_add(dst_bf, t1[:], t2[:])

        def make_q_fillers(hq):
            """q proj for head hq: per-group accumulate (1 PSUM bank),
            2 MMs per filler unit, rope after each group."""
            units = []
            state = {}

            def proj(g, k):
                def fn():
                    if k == 0:
                        state[g] = q2ps.tile([128, GRP], F32, tag="q2",
                                             name=f"q2g{g}")
                    for kk in (k, k + 1):
                        nc.tensor.matmul(
                            state[g][:], wq_sb[:, hq, kk, :],
                            hs_sb[:, kk, g * GRP:(g + 1) * GRP],
                            start=(kk == 0), stop=(kk == N_HK - 1),
                        )
                return fn

            def rope_g(g):
                def fn():
                    rope(qt_sb[hq][:, g * GRP:(g + 1) * GRP],
                         state[g], cosq_sb, sinq_sb, g)
                return fn

            for g in range(N_G):
                for k in range(0, N_HK, 2):
                    units.append(proj(g, k))
                units.append(rope_g(g))
            return units

        # ---------------- pass 1: q0 / k / v projections ----------------
        p1_stack = ExitStack()
        p1ps = p1_stack.enter_context(
            tc.tile_pool(name="p1ps", bufs=1, space="PSUM"))
        vtpool = p1_stack.enter_context(tc.tile_pool(name="vt", bufs=2))

        q0_ps = [p1ps.tile([128, GRP], F32, tag=f"q0g{g}", name=f"q0g{g}")
                 for g in range(N_G)]
        k_ps = [p1ps.tile([128, GRP], F32, tag=f"kg{g}", name=f"kg{g}")
                for g in range(N_G)]
        v_ps = [p1ps.tile([128, GRP], F32, tag=f"vg{g}", name=f"vg{g}")
                for g in range(N_G)]
        for k in range(N_HK):
            first, last = (k == 0), (k == N_HK - 1)
            for dst, lhsT in (
                (q0_ps, wq_sb[:, 0, k, :]),
                (k_ps, wkv_sb[:, k, 0:128]),
                (v_ps, wkv_sb[:, k, 128:256]),
            ):
                for g in range(N_G):
                    nc.tensor.matmul(
                        dst[g][:], lhsT, hs_sb[:, k, g * GRP:(g + 1) * GRP],
                        start=first, stop=last,
                    )
        # ropes: k first (needed by every head), then q0; head-1 q-proj
        # filler units interleaved so the PE is not starved by the rope's
        # ACT/DVE chain
        fillers0 = make_q_fillers(1)
        for g in range(N_G):
            rope(kt_sb[:, PAST + g * GRP: PAST + (g + 1) * GRP],
                 k_ps[g], cosk_sb, sink_sb, g)
            for _ in range(3):
                if fillers0:
                    fillers0.pop(0)()
        for g in range(N_G):
            rope(qt_sb[0][:, g * GRP:(g + 1) * GRP], q0_ps[g], cosq_sb, sinq_sb, g)
            for _ in range(3):
                if fillers0:
                    fillers0.pop(0)()
        # v: [d, s] -> bf16 staging -> xbar-transpose into v_sb [s, d] slots
        # (emitted after the ropes so the q_f ACT copies are not queued
        # behind the vt copies; v tiles 24+ are first read ~3us into head 0)
        for g in range(N_G):
            vt_sb = vtpool.tile([128, GRP], BF16, tag="vt", name="vt_sb")
            nc.scalar.copy(vt_sb[:], v_ps[g][:])
            for j in range(4):
                slot = N_PV + g * 4 + j
                nc.sync.dma_start(
                    v_sb[:, slot * 128:(slot + 1) * 128],
                    vt_sb[:, j * 128:(j + 1) * 128],
                    transpose=True,
                )
        p1_stack.close()
        p1res_stack.close()

        # ---------------- stage 2: attention w/ fillers ----------------
        s2_stack = ExitStack()
        scps = s2_stack.enter_context(
            tc.tile_pool(name="scps", bufs=2, space="PSUM"))
        aps = s2_stack.enter_context(
            tc.tile_pool(name="aps", bufs=1, space="PSUM"))
        ptpool = s2_stack.enter_context(tc.tile_pool(name="pt", bufs=6))
        smpool = s2_stack.enter_context(tc.tile_pool(name="softm", bufs=2))
        smpool1 = s2_stack.enter_context(tc.tile_pool(name="softm1", bufs=1))

        # stage-3 prefix PSUM: pool opened here (below q2 in the stack) but
        # its tiles are only allocated during head 3, after q2ps closes and
        # frees the banks
        N_PRE = 1
        pre_stack = ExitStack()
        ostpool = pre_stack.enter_context(tc.tile_pool(name="ostage", bufs=2))

        pre_state = {}

        deferred_tail = [[]]


        def make_pre_fillers():
            """stage-3 prefix: first N_PRE o_proj chunks' h0/h1 matmuls.
            (only heads 0/1 - their at_sb are ready long before head 3's
            tail; h2/h3 + copies are completed after the head-3 tail)."""
            units = []

            def pre(j):
                def fn():
                    o_ps = q2ps.tile([128, 512], F32, tag="q2", name="o_pre")
                    pre_state[j] = o_ps
                    for hh in range(2):
                        nc.tensor.matmul(
                            o_ps[:],
                            at_sb[hh][:, 0:128],
                            wo_sb[:, hh * HID + j * 512:hh * HID + (j + 1) * 512],
                            start=(hh == 0), stop=False,
                        )
                return fn

            for j in range(N_PRE):
                units.append(pre(j))
            return units

        for h in range(HPC):
            if h == 0:
                fillers = fillers0
            elif h < HPC - 1:
                fillers = make_q_fillers(h + 1)
            else:
                fillers = make_pre_fillers()

            a_ps = aps.tile([128, Q], F32, tag="aacc", name="a_ps")
            dn0 = smpool.tile([128, Q], BF16, tag="dn0", name="dn0")
            dn1 = smpool.tile([128, Q], BF16, tag="dn1", name="dn1")
            for _ in range(4):
                if fillers:
                    fillers.pop(0)()
            pend = []
            touched = set()

            def emit_attn(prev):
                jj, pt = prev
                qs = 0 if jj < 24 else (jj - 24) * 128
                spans = [(qs, GRP), (GRP, Q)] if qs < GRP else [(qs, Q)]
                for a, b in spans:
                    nc.tensor.matmul(
                        a_ps[:, a:b],
                        v_sb[:, jj * 128:(jj + 1) * 128],
                        pt[:, a:b],
                        start=(jj == ATTN_FIRST), stop=(jj == ATTN_LAST),
                    )

            for pos, jj in enumerate(ORDER):
                s_ps = scps.tile([128, Q], F32, tag="ss", name="s_ps")
                pt = ptpool.tile([128, Q], BF16, tag="pt", name="pt")
                # exact causal window: kv tile jj sees q in [qs, Q)
                qs = 0 if jj < 24 else (jj - 24) * 128
                if qs < GRP:
                    spans = [(qs, GRP), (GRP, Q)] if qs < GRP else []
                else:
                    spans = [(qs, Q)]
                if qs == 0:
                    spans = [(0, GRP), (GRP, Q)]
                for a, b in spans:
                    nc.tensor.matmul(
                        s_ps[:, a:b],
                        kt_sb[:, jj * 128:(jj + 1) * 128],
                        qt_sb[h][:, a:b],
                        start=True, stop=True,
                    )
                nc.scalar.activation(
                    pt[:, qs:Q], s_ps[:, qs:Q],
                    mybir.ActivationFunctionType.Exp,
                    bias=shift_sb[:], scale=1.0,
                )
                # diagonal block masking: identical lower-triangular
                # [128,128] pattern for every jj >= 24, at cols [qs, qs+128)
                if jj >= 24:
                    nc.vector.tensor_mul(
                        pt[:, qs:qs + 128], pt[:, qs:qs + 128], tri_sb[:])
                # denominator accumulation (DVE, 2 bf16 accumulators)
                par = pos % 2
                dn = dn0 if par == 0 else dn1
                if par not in touched:
                    touched.add(par)
                    nc.vector.tensor_copy(dn[:], pt[:])
                else:
                    deng = (nc.gpsimd if pos == len(ORDER) - 1
                            else nc.vector)
                    deng.tensor_add(dn[:, qs:Q], dn[:, qs:Q], pt[:, qs:Q])

                pend.append((jj, pt))
                if len(pend) > 2:
                    emit_attn(pend.pop(0))
                if pos in (3, 9) and deferred_tail[0]:
                    deferred_tail[0].pop(0)()
                npop = 2 if pos < 8 else 1
                for _ in range(npop):
                    if fillers:
                        fillers.pop(0)()
            for ent in pend:
                emit_attn(ent)
            while fillers:
                fillers.pop(0)()
            # unnormalized attn out (frees a_ps for the next head)
            au_sb = smpool1.tile([128, Q], BF16, tag="atu", name="au_sb")
            nc.vector.tensor_copy(au_sb[:, 0:GRP], a_ps[:, 0:GRP])
            nc.scalar.copy(au_sb[:, GRP:Q], a_ps[:, GRP:Q])

            def make_tail(h, au_sb, dn0, dn1):
                rc_sb = smpool1.tile([2, GRP], BF16, tag="recip", name="rc_sb")

                def tail_ds():
                    ds_t = auxps.tile([128, GRP], F32, tag="aux", name="ds_t")
                    ds_ps = ds_t[0:2, :]
                    for idx, (sel, dn, hoff) in enumerate(
                        [(ones2a, dn0, 0), (ones2a, dn1, 0),
                         (ones2b, dn0, GRP), (ones2b, dn1, GRP)]
                    ):
                        nc.tensor.matmul(
                            ds_ps[:], sel[:], dn[:, hoff:hoff + GRP],
                            start=(idx == 0), stop=(idx == 3),
                        )
                    with nc.allow_low_precision(reason="1/denom bf16"):
                        nc.vector.reciprocal(rc_sb[:], ds_ps[:])

                def tail_bc():
                    for half in range(2):
                        hsl = slice(half * GRP, (half + 1) * GRP)
                        bc_ps = auxps.tile([128, GRP], F32, tag="aux",
                                           name="bc_ps")
                        nc.tensor.matmul(
                            bc_ps[:], sel_a if half == 0 else sel_b,
                            rc_sb[:], start=True, stop=True)
                        nc.vector.tensor_mul(
                            at_sb[h][:, hsl], au_sb[:, hsl], bc_ps[:])
                return [tail_ds, tail_bc]

            deferred_tail[0] = make_tail(h, au_sb, dn0, dn1)
        for fn in deferred_tail[0]:
            fn()

        # ---------------- stage 3: o_proj partial ----------------
        def finish_chunk(st_i, c, o_ps, h_start, opool):
            for hh in range(h_start, HPC):
                nc.tensor.matmul(
                    o_ps[:],
                    at_sb[hh][:, st_i * 128:(st_i + 1) * 128],
                    wo_sb[:, hh * HID + c * 512:hh * HID + (c + 1) * 512],
                    start=(hh == 0), stop=(hh == HPC - 1),
                )
            o_sb = opool.tile([128, 512], BF16, tag="osb", name="o_sb")
            if st_i == 7 and c >= 6:
                # final two chunks: split the copy across both engines and
                # the store into two parallel DMAs to halve the drain tail
                nc.scalar.copy(o_sb[:, 0:256], o_ps[:, 0:256])
                nc.vector.tensor_copy(o_sb[:, 256:512], o_ps[:, 256:512])
                for hf in range(2):
                    nc.sync.dma_start(
                        outp[st_i * 128:(st_i + 1) * 128,
                             c * 512 + hf * 256:c * 512 + (hf + 1) * 256],
                        o_sb[:, hf * 256:(hf + 1) * 256],
                    )
                return
            if (st_i + c) % 2 == 0:
                nc.scalar.copy(o_sb[:], o_ps[:])
            else:
                nc.vector.tensor_copy(o_sb[:], o_ps[:])
            nc.sync.dma_start(
                outp[st_i * 128:(st_i + 1) * 128, c * 512:(c + 1) * 512],
                o_sb[:],
            )

        # complete the prefix chunks (st 0, c 0..N_PRE-1; h0/h1 already
        # accumulated during head 3), then release pools bottom-up and run
        # the remaining chunks with full PSUM
        for j in range(N_PRE):
            finish_chunk(0, j, pre_state[j], 2, ostpool)
        pre_stack.close()
        s2_stack.close()
        with (
            tc.tile_pool(name="ops", bufs=4, space="PSUM") as opps,
            tc.tile_pool(name="ostage2", bufs=4) as ostpool2,
        ):
            for st_i in range(8):
                for c in range(8):
                    if st_i == 0 and c < N_PRE:
                        continue
                    o_ps = opps.tile([128, 512], F32, tag="ops", name="o_ps")
                    finish_chunk(st_i, c, o_ps, 0, ostpool2)
        q2_stack.close()
        rot_stack.close()
    return nc


class TileCtx:
    """TileContext plus an ExitStack that closes before the context exits."""

    def __init__(self, nc):
        self.nc = nc

    def __enter__(self):
        self.tc = tile.TileContext(self.nc)
        self.tc.__enter__()
        self.st = ExitStack()
        return self.tc, self.st

    def __exit__(self, *exc):
        self.st.close()
        return self.tc.__exit__(*exc)


def _pack_ktiles(a, tile_rows=128):
    """[R, C] -> [128, (R//128)*C] with k-tile kt at cols [kt*C:(kt+1)*C]."""
    r, c = a.shape
    n = r // tile_rows
    return np.ascontiguousarray(
        a.reshape(n, tile_rows, c).transpose(1, 0, 2).reshape(tile_rows, n * c)
    )


def _rope_tables_ds(position_ids):
    """cos/sin in [d, s] layout: [128, Q] f64."""
    pos = np.asarray(position_ids).reshape(-1).astype(np.float64)      # [Q]
    inv_freq = 1.0 / (ROPE_THETA ** (np.arange(0, HD, 2, dtype=np.float64) / HD))
    ang_half = np.outer(inv_freq, pos)                                 # [64, Q]
    ang = np.concatenate([ang_half, ang_half], axis=0)                 # [128, Q]
    return np.cos(ang), np.sin(ang)


def kernel(hidden_states, attention_mask, position_ids, past_k, past_v,
           Wq, Wk, Wv, Wo):
    global LAST_RESULTS
    bf = ml_dtypes.bfloat16

    hs = np.asarray(hidden_states, np.float32).reshape(Q, HID)
    mask = np.asarray(attention_mask, np.float32).reshape(Q, KV)
    cos_d, sin_d = _rope_tables_ds(position_ids)

    scale = 1.0 / math.sqrt(HD)
    cosq_t = (cos_d * scale).astype(bf)
    sinq_t = (sin_d * scale).astype(bf)
    cosk_t = cos_d.astype(bf)
    sink_t = sin_d.astype(bf)

    # rotate-half permutation with sign: rot[d] = -x[d+64] (d<64); x[d-64]
    prot_np = np.zeros((128, 128), np.float32)
    for dd in range(64):
        prot_np[dd + 64, dd] = -1.0
        prot_np[dd, dd + 64] = 1.0
    prot_t = prot_np.astype(bf)

    # diagonal masks: [128 kv, 4 tiles * 512 q]: kv tile 24+m vs queries
    # 0..511 (identical pattern to kv tile 28+m vs queries 512..1023)
    mask_t = np.empty((128, 2048), np.float32)
    for m in range(4):
        kt = 24 + m
        blk = mask[0:512, kt * 128:(kt + 1) * 128].T
        mask_t[:, m * 512:(m + 1) * 512] = (blk == 0.0).astype(np.float32)
    mask_t = mask_t.astype(bf)
    # lower-triangular [128,128] diagonal-block mask (kv_sub <= q_sub),
    # derived from the input mask at (q 0:128, kv 3072:3200)
    tri_t = np.ascontiguousarray(
        (mask[0:128, PAST:PAST + 128] == 0.0).astype(np.float32).T).astype(bf)

    sel2_np = np.zeros((2, 256), np.float32)
    sel2_np[0, 0:128] = 1.0      # sel_a: broadcast rc row 0
    sel2_np[1, 128:256] = 1.0    # sel_b: broadcast rc row 1

    hst = _pack_ktiles(np.ascontiguousarray(hs.T)).astype(bf)
    hst = hst.reshape(128, N_HK, Q)

    nc = _build_program()
    in_maps = []
    for c in range(NCORES):
        ks = slice(c * HD, (c + 1) * HD)
        wq_heads = []
        for h in range(HPC):
            rows = slice((HPC * c + h) * HD, (HPC * c + h + 1) * HD)
            wq_heads.append(
                _pack_ktiles(np.ascontiguousarray(Wq[rows, :].T))
                .reshape(128, N_HK, 128)
            )
        wq_c = np.ascontiguousarray(
            np.stack(wq_heads, axis=1)).astype(bf)                 # [128,4,32,128]
        wk_c = np.ascontiguousarray(Wk[ks, :].T)                   # [4096, 128]
        wv_c = np.ascontiguousarray(Wv[ks, :].T)
        wkv_c = _pack_ktiles(
            np.concatenate([wk_c, wv_c], axis=1)
        ).astype(bf).reshape(128, N_HK, 256)
        pkt = np.ascontiguousarray(past_k[0, c].T).astype(bf)      # [128, 3072]
        pv = _pack_ktiles(np.ascontiguousarray(past_v[0, c])).astype(bf)
        qs = slice(c * HPC * HD, (c + 1) * HPC * HD)
        wo_c = _pack_ktiles(
            np.ascontiguousarray(Wo[:, qs].T)).astype(bf)          # [128, 4*4096]
        in_maps.append({
            "hst": hst, "wqt": wq_c, "wkvt": wkv_c, "pastkt": pkt,
            "pastv": pv, "maskt": mask_t, "cosq": cosq_t, "sinq": sinq_t,
            "cosk": cosk_t, "sink": sink_t, "prot": prot_t, "trit": tri_t,
            "sel2": sel2_np.astype(bf),
            "wot": wo_c,
        })

    res = run_bass_kernel_spmd(nc, in_maps, list(range(NCORES)))
    LAST_RESULTS = res
    out = np.zeros((Q, HID), np.float32)
    for c in range(NCORES):
        out += np.asarray(res.results[c]["outp"], dtype=np.float32)
    return out.reshape(B, Q, HID)


# revision 38
# speedup vs baseline: 1.0169x; 1.0169x over previous
"""Llama GQA attention (B=1, Q=1024, PAST=3072, HID=4096, NH=32, NKV=8, HD=128)
tensor-parallel over heads across 8 NeuronCores.

Per core c: kv head c, query heads 4c..4c+3. Each core computes its partial
o_proj contribution [1024, 4096] in bf16; the host sums the 8 partials in f32.

v3 layout (vs v2): restructured so the ScalarE exp stream (the stage-2
bottleneck, ~35us/head) starts ~50us earlier and the PE never starves:
  - pass 1 computes ONLY q head 0 + k + v (6 N=512 MMs per hid k-tile,
    DMA-paced); head 0's attention starts right after the k/q0 ropes.
  - q proj for head h+1 runs as PE *filler* inside head h's attention loop
    (the attention loop is exp-bound on ScalarE, leaving PE idle slots).
  - v proj is W-stationary (out [d,s], 32 N=512 MMs per group instead of
    128 N=128 hs-stationary MMs); [s,d] layout for attention is recovered
    with 8 SBUF->SBUF xbar DMA transposes (off-engine).
  - scores in f32 PSUM [128,1024] supertiles (1 kv tile x full q), exp is
    one ACTIVATE per supertile; diagonal masking via 0/1 mask multiplies
    split GpSimd/DVE (as v2).
  - softmax tail: denominator partition-reduce via ones-matmuls into a
    [2,512] PSUM tile, reciprocal_approx_fast (0.9us vs 3.3us iterative
    divide), broadcast matmul result consumed directly from PSUM by the
    normalize multiply (no bc copy). Tail is deferred into the next head.
  - o_proj: [128,512] chunks, h-accumulated in PSUM; the first chunks' h0/h1
    matmuls run as head-3 filler; exp table preloaded at t=0.
"""

import math
import os as _os
import numpy as np
import ml_dtypes
from contextlib import ExitStack

import bass_rust
import concourse.bass as bass
import concourse.mybir as mybir
import concourse.tile as tile
from concourse.vector_clock import ScopedClock
from concourse.bass_utils import run_bass_kernel_spmd

# ---------------------------------------------------------------------------
# Workaround: walrus in this image rejects >1 sem wait on CTRL-class
# instructions (Drain/NoOp). TileContext's tail drain waits on every touched
# logical processor. Split the waits across preceding sync-engine nops.
MAX_WAITS = 1


def _split_waits(nc, inst):
    si = inst.ins.sync_info
    if si is None:
        return
    waits = list(si.on_wait)
    if len(waits) <= MAX_WAITS:
        return
    inst.ins.sync_info = bass_rust.SyncInfo(
        on_wait=waits[:MAX_WAITS], on_update=list(si.on_update)
    )
    rest = waits[MAX_WAITS:]
    while rest:
        extra = nc.sync.nop(nofuse=True)
        extra.ins.sync_info = bass_rust.SyncInfo(on_wait=rest[:MAX_WAITS], on_update=[])
        rest = rest[MAX_WAITS:]


def _drain_and_barrier_split(self, tick_clock, wait_clock):
    nc = self.nc
    carrier = nc.sync.nop(nofuse=True)
    wait_clock.add_sem_waits(carrier.ins, ScopedClock({None: tick_clock.global_clock}))
    _split_waits(nc, carrier)
    nc.sync.drain()
    nc.all_engine_barrier()
    popped = nc._tile_sem_poison_stack.pop()
    assert popped is self._sem_poison
    nc.clear_and_free_semaphores(list(self.sems.allocated().values()))
    nc.all_engine_barrier()


tile.TileContext._drain_and_barrier = _drain_and_barrier_split
# ---------------------------------------------------------------------------

# ---------------------------------------------------------------------------
# General wait-cap legalization: this walrus rejects instructions carrying
# more than a couple of sem waits. Post-process the BIR JSON: hoist overflow
# waits onto engine-matched NoOps inserted immediately before the offender
# (same engine queue -> same ordering semantics).
import json as _json

_CTRL_OPS = {"NoOp", "Drain", "EventSemaphore"}
_CAP_CTRL = 1
_CAP_OTHER = 1
if not hasattr(bass.Bass, "_orig_to_json_bytes"):
    bass.Bass._orig_to_json_bytes = bass.Bass.to_json_bytes
_orig_to_json_bytes = bass.Bass._orig_to_json_bytes


def _legalized_to_json_bytes(self, *a, **k):
    raw = _orig_to_json_bytes(self, *a, **k)
    m = _json.loads(raw)
    ctr = [0]
    changed = False
    for fn in m.get("functions", []):
        for blk in fn.get("blocks", []):
            insts = blk.get("instructions", [])
            out = []
            for ins in insts:
                si = ins.get("sync_info")
                if si:
                    waits = si.get("on_wait") or []
                    cap = _CAP_CTRL if ins.get("opcode") in _CTRL_OPS else _CAP_OTHER
                    if len(waits) > cap:
                        changed = True
                        rest = waits[:-cap]
                        si["on_wait"] = waits[-cap:]
                        while rest:
                            ctr[0] += 1
                            out.append({
                                "debug": ins.get("debug", 0),
                                "engine": ins["engine"],
                                "ins": [], "outs": [],
                                "name": f"{ins['name']}_lw{ctr[0]}",
                                "opcode": "NoOp",
                                "sync_info": {"on_wait": rest[:_CAP_CTRL],
                                              "on_update": []},
                            })
                            rest = rest[_CAP_CTRL:]
                out.append(ins)
            blk["instructions"] = out
    if not changed:
        return raw
    return _json.dumps(m).encode()


bass.Bass.to_json_bytes = _legalized_to_json_bytes
# ---------------------------------------------------------------------------


B, Q, PAST, HID = 1, 1024, 3072, 4096
NH, NKV, HD = 32, 8, 128
KV = PAST + Q           # 4096
NCORES = 8
HPC = NH // NCORES      # 4 query heads per core
ROPE_THETA = 10000.0
EXP_SHIFT = -20.0       # constant softmax shift (cancels exactly per row)

F32 = mybir.dt.float32
BF16 = mybir.dt.bfloat16

N_KT = KV // 128        # 32 kv tiles
N_HK = HID // 128       # 32 hid k-tiles
GRP = 512               # query group width
N_G = Q // GRP          # 2 groups
N_PV = PAST // 128      # 24 past-v tiles

# attention supertile order: 1 kv tile x full q (narrow tiles 28-31 only vs
# q 512:1024). Past tiles first so the attn-start flags never wait on new kv;
# diagonal/narrow (masked) tiles interleaved 1:1 to spread the mask muls.
ORDER = [0, 24, 1, 25, 2, 26, 3, 27, 4, 28, 5, 29, 6, 30, 7, 31] + list(range(8, 24))
ATTN_FIRST, ATTN_LAST = 0, 23

LAST_RESULTS = None     # test harness reads exec_time_ns from here


def _build_program():
    nc = bass.Bass()
    hst = nc.declare_dram_parameter("hst", [128, N_HK, Q], BF16, isOutput=False)
    wqt = nc.declare_dram_parameter("wqt", [128, HPC, N_HK, 128], BF16, isOutput=False)
    wkvt = nc.declare_dram_parameter("wkvt", [128, N_HK, 256], BF16, isOutput=False)
    pastkt = nc.declare_dram_parameter("pastkt", [128, PAST], BF16, isOutput=False)
    pastv = nc.declare_dram_parameter("pastv", [128, PAST], BF16, isOutput=False)
    maskt = nc.declare_dram_parameter("maskt", [128, 2048], BF16, isOutput=False)
    trit = nc.declare_dram_parameter("trit", [128, 128], BF16, isOutput=False)
    # rope tables in [d, seq] layout; q tables pre-scaled by 1/sqrt(HD)
    cosq = nc.declare_dram_parameter("cosq", [128, Q], BF16, isOutput=False)
    sinq = nc.declare_dram_parameter("sinq", [128, Q], BF16, isOutput=False)
    cosk = nc.declare_dram_parameter("cosk", [128, Q], BF16, isOutput=False)
    sink = nc.declare_dram_parameter("sink", [128, Q], BF16, isOutput=False)
    prot = nc.declare_dram_parameter("prot", [128, 128], BF16, isOutput=False)
    sel2 = nc.declare_dram_parameter("sel2", [2, 256], BF16, isOutput=False)
    wot = nc.declare_dram_parameter("wot", [128, HPC * HID], BF16, isOutput=False)
    outp = nc.declare_dram_parameter("outp", [Q, HID], BF16, isOutput=True)

    with TileCtx(nc) as (tc, st):
        cpool = st.enter_context(tc.tile_pool(name="const", bufs=1))
        kvpool = st.enter_context(tc.tile_pool(name="kvres", bufs=1))
        qtpool = st.enter_context(tc.tile_pool(name="qt", bufs=1))
        apool = st.enter_context(tc.tile_pool(name="attn", bufs=1))
        hspool = st.enter_context(tc.tile_pool(name="hsw", bufs=1))
        wopool = st.enter_context(tc.tile_pool(name="wo", bufs=1))

        # ---- consts / residents ----
        ones2a = cpool.tile([128, 2], BF16)
        nc.vector.memset(ones2a[:], 0.0)
        nc.vector.memset(ones2a[:, 0:1], 1.0)
        ones2b = cpool.tile([128, 2], BF16)
        nc.vector.memset(ones2b[:], 0.0)
        nc.vector.memset(ones2b[:, 1:2], 1.0)
        sel2_sb = cpool.tile([2, 256], BF16)
        sel_a = sel2_sb[:, 0:128]
        sel_b = sel2_sb[:, 128:256]
        shift_sb = cpool.tile([128, 1], F32)
        nc.vector.memset(shift_sb[:], EXP_SHIFT)
        prot_sb = cpool.tile([128, 128], BF16)
        mask_sb = cpool.tile([128, 2048], BF16)
        tri_sb = cpool.tile([128, 128], BF16)

        # exp table preload: hide the ~2.7us ACT_TABLE_LOAD under startup DMA
        warm_i = cpool.tile([128, 1], F32)
        nc.vector.memset(warm_i[:], 0.0)
        warm_o = cpool.tile([128, 1], BF16)
        nc.scalar.activation(warm_o[:], warm_i[:],
                             mybir.ActivationFunctionType.Exp,
                             bias=shift_sb[:], scale=1.0)

        kt_sb = kvpool.tile([128, KV], BF16)
        v_sb = kvpool.tile([128, N_KT * 128], BF16)
        qt_sb = [qtpool.tile([128, Q], BF16, tag=f"qt{h}", name=f"qt{h}")
                 for h in range(HPC)]
        at_sb = [apool.tile([128, Q], BF16, tag=f"at{h}", name=f"at{h}")
                 for h in range(HPC)]

        hs_sb = hspool.tile([128, N_HK, Q], BF16)
        wq_sb = hspool.tile([128, HPC, N_HK, 128], BF16)
        cosq_sb = hspool.tile([128, Q], BF16)
        sinq_sb = hspool.tile([128, Q], BF16)
        wo_sb = wopool.tile([128, HPC * HID], BF16)
        # shared aux PSUM (rope rot + softmax tail, temporally disjoint):
        # one [128,512]-f32 bank ring; opened first, closed last
        rot_stack = ExitStack()
        auxps = rot_stack.enter_context(
            tc.tile_pool(name="auxps", bufs=1, space="PSUM"))
        rpool = rot_stack.enter_context(tc.tile_pool(name="rope", bufs=2))
        q2_stack = ExitStack()
        q2ps = q2_stack.enter_context(
            tc.tile_pool(name="q2ps", bufs=1, space="PSUM"))
        # pass-1-only residents (freed before the stage-2 pools allocate)
        p1res_stack = ExitStack()
        p1res = p1res_stack.enter_context(tc.tile_pool(name="p1res", bufs=1))
        wkv_sb = p1res.tile([128, N_HK, 256], BF16)
        cosk_sb = p1res.tile([128, Q], BF16)
        sink_sb = p1res.tile([128, Q], BF16)

        # ---- DMA issue order = consumption order ----
        # pass-1-critical: hs + wq(head0) + wkv, finely chunked up front
        bounds = [0, 1, 2, 3, 4, 5, 6, 8, 10, 12, 14, 16, 20, 24, 28, 32]
        for i in range(len(bounds) - 1):
            s, e = bounds[i], bounds[i + 1]
            nc.sync.dma_start(hs_sb[:, s:e, :], hst[:, s:e, :])
            nc.sync.dma_start(wq_sb[:, 0, s:e, :], wqt[:, 0, s:e, :])
            nc.sync.dma_start(wkv_sb[:, s:e, :], wkvt[:, s:e, :])
        # rope tables + consts (needed ~50us)
        nc.sync.dma_start(cosq_sb[:], cosq[:])
        nc.sync.dma_start(sinq_sb[:], sinq[:])
        nc.sync.dma_start(cosk_sb[:], cosk[:])
        nc.sync.dma_start(sink_sb[:], sink[:])
        nc.sync.dma_start(prot_sb[:], prot[:])
        nc.sync.dma_start(mask_sb[:], maskt[:])
        nc.sync.dma_start(tri_sb[:], trit[:])
        nc.sync.dma_start(sel2_sb[:], sel2[:])
        # past kv (needed at head-0 attention ~55us)
        nc.sync.dma_start(kt_sb[:, :PAST], pastkt[:])
        nc.sync.dma_start(v_sb[:, : N_PV * 128], pastv[:])
        # filler q weights (heads 1-3, needed from ~55us)
        for h in range(1, HPC):
            nc.sync.dma_start(wq_sb[:, h, :, :], wqt[:, h, :, :])
        # wo last (needed ~220us)
        for h in range(HPC):
            nc.sync.dma_start(
                wo_sb[:, h * HID:(h + 1) * HID], wot[:, h * HID:(h + 1) * HID]
            )

        def rope(dst_bf, src_ps, cos_t, sin_t, g):
            """dst_bf [128 d, 512 s] <- RoPE applied in [d, s] layout."""
            c = cos_t[:, g * GRP:(g + 1) * GRP]
            s = sin_t[:, g * GRP:(g + 1) * GRP]
            q_f = rpool.tile([128, GRP], BF16, tag="qf", name="q_f")
            nc.scalar.copy(q_f[:], src_ps[:])
            rot_ps = auxps.tile([128, GRP], F32, tag="aux", name="rot_ps")
            nc.tensor.matmul(rot_ps[:], prot_sb[:], q_f[:], start=True, stop=True)
            t1 = rpool.tile([128, GRP], F32, tag="t1", name="t1")
            nc.vector.tensor_mul(t1[:], src_ps[:], c)
            t2 = rpool.tile([128, GRP], F32, tag="t2", name="t2")
            nc.vector.tensor_mul(t2[:], rot_ps[:], s)
            nc.vector.tensor_add(dst_bf, t1[:], t2[:])

        def make_q_fillers(hq):
            """q proj for head hq: per-group accumulate (1 PSUM bank),
            2 MMs per filler unit, rope after each group."""
            units = []
            state = {}

            def proj(g, k):
                def fn():
                    if k == 0:
                        state[g] = q2ps.tile([128, GRP], F32, tag="q2",
                                             name=f"q2g{g}")
                    for kk in (k, k + 1):
                        nc.tensor.matmul(
                            state[g][:], wq_sb[:, hq, kk, :],
                            hs_sb[:, kk, g * GRP:(g + 1) * GRP],
                            start=(kk == 0), stop=(kk == N_HK - 1),
                        )
                return fn

            def rope_g(g):
                def fn():
                    rope(qt_sb[hq][:, g * GRP:(g + 1) * GRP],
                         state[g], cosq_sb, sinq_sb, g)
                return fn

            for g in range(N_G):
                for k in range(0, N_HK, 2):
                    units.append(proj(g, k))
                units.append(rope_g(g))
            return units

        # ---------------- pass 1: q0 / k / v projections ----------------
        p1_stack = ExitStack()
        p1ps = p1_stack.enter_context(
            tc.tile_pool(name="p1ps", bufs=1, space="PSUM"))
        vtpool = p1_stack.enter_context(tc.tile_pool(name="vt", bufs=2))

        q0_ps = [p1ps.tile([128, GRP], F32, tag=f"q0g{g}", name=f"q0g{g}")
                 for g in range(N_G)]
        k_ps = [p1ps.tile([128, GRP], F32, tag=f"kg{g}", name=f"kg{g}")
                for g in range(N_G)]
        v_ps = [p1ps.tile([128, GRP], F32, tag=f"vg{g}", name=f"vg{g}")
                for g in range(N_G)]
        for k in range(N_HK):
            first, last = (k == 0), (k == N_HK - 1)
            for dst, lhsT in (
                (q0_ps, wq_sb[:, 0, k, :]),
                (k_ps, wkv_sb[:, k, 0:128]),
                (v_ps, wkv_sb[:, k, 128:256]),
            ):
                for g in range(N_G):
                    nc.tensor.matmul(
                        dst[g][:], lhsT, hs_sb[:, k, g * GRP:(g + 1) * GRP],
                        start=first, stop=last,
                    )
        # ropes: k first (needed by every head), then q0; head-1 q-proj
        # filler units interleaved so the PE is not starved by the rope's
        # ACT/DVE chain
        fillers0 = make_q_fillers(1)
        for g in range(N_G):
            rope(kt_sb[:, PAST + g * GRP: PAST + (g + 1) * GRP],
                 k_ps[g], cosk_sb, sink_sb, g)
            for _ in range(3):
                if fillers0:
                    fillers0.pop(0)()
        for g in range(N_G):
            rope(qt_sb[0][:, g * GRP:(g + 1) * GRP], q0_ps[g], cosq_sb, sinq_sb, g)
            for _ in range(3):
                if fillers0:
                    fillers0.pop(0)()
        # v: [d, s] -> bf16 staging -> xbar-transpose into v_sb [s, d] slots
        # (emitted after the ropes so the q_f ACT copies are not queued
        # behind the vt copies; v tiles 24+ are first read ~3us into head 0)
        for g in range(N_G):
            vt_sb = vtpool.tile([128, GRP], BF16, tag="vt", name="vt_sb")
            nc.scalar.copy(vt_sb[:], v_ps[g][:])
            for j in range(4):
                slot = N_PV + g * 4 + j
                nc.sync.dma_start(
                    v_sb[:, slot * 128:(slot + 1) * 128],
                    vt_sb[:, j * 128:(j + 1) * 128],
                    transpose=True,
                )
        p1_stack.close()
        p1res_stack.close()

        # ---------------- stage 2: attention w/ fillers ----------------
        s2_stack = ExitStack()
        scps = s2_stack.enter_context(
            tc.tile_pool(name="scps", bufs=2, space="PSUM"))
        aps = s2_stack.enter_context(
            tc.tile_pool(name="aps", bufs=1, space="PSUM"))
        ptpool = s2_stack.enter_context(tc.tile_pool(name="pt", bufs=6))
        smpool = s2_stack.enter_context(tc.tile_pool(name="softm", bufs=2))
        smpool1 = s2_stack.enter_context(tc.tile_pool(name="softm1", bufs=1))

        # stage-3 prefix PSUM: pool opened here (below q2 in the stack) but
        # its tiles are only allocated during head 3, after q2ps closes and
        # frees the banks
        N_PRE = 1
        pre_stack = ExitStack()
        ostpool = pre_stack.enter_context(tc.tile_pool(name="ostage", bufs=2))

        pre_state = {}

        deferred_tail = [[]]


        def make_pre_fillers():
            """stage-3 prefix: first N_PRE o_proj chunks' h0/h1 matmuls.
            (only heads 0/1 - their at_sb are ready long before head 3's
            tail; h2/h3 + copies are completed after the head-3 tail)."""
            units = []

            def pre(j):
                def fn():
                    o_ps = q2ps.tile([128, 512], F32, tag="q2", name="o_pre")
                    pre_state[j] = o_ps
                    for hh in range(2):
                        nc.tensor.matmul(
                            o_ps[:],
                            at_sb[hh][:, 0:128],
                            wo_sb[:, hh * HID + j * 512:hh * HID + (j + 1) * 512],
                            start=(hh == 0), stop=False,
                        )
                return fn

            for j in range(N_PRE):
                units.append(pre(j))
            return units

        for h in range(HPC):
            if h == 0:
                fillers = fillers0
            elif h < HPC - 1:
                fillers = make_q_fillers(h + 1)
            else:
                fillers = make_pre_fillers()

            a_ps = aps.tile([128, Q], F32, tag="aacc", name="a_ps")
            dn0 = smpool.tile([128, Q], BF16, tag="dn0", name="dn0")
            dn1 = smpool.tile([128, Q], BF16, tag="dn1", name="dn1")
            for _ in range(4):
                if fillers:
                    fillers.pop(0)()
            pend = []
            touched = set()

            def emit_attn(prev):
                jj, pt = prev
                qs = 0 if jj < 24 else (jj - 24) * 128
                spans = [(qs, GRP), (GRP, Q)] if qs < GRP else [(qs, Q)]
                for a, b in spans:
                    nc.tensor.matmul(
                        a_ps[:, a:b],
                        v_sb[:, jj * 128:(jj + 1) * 128],
                        pt[:, a:b],
                        start=(jj == ATTN_FIRST), stop=(jj == ATTN_LAST),
                    )

            for pos, jj in enumerate(ORDER):
                s_ps = scps.tile([128, Q], F32, tag="ss", name="s_ps")
                pt = ptpool.tile([128, Q], BF16, tag="pt", name="pt")
                # exact causal window: kv tile jj sees q in [qs, Q)
                qs = 0 if jj < 24 else (jj - 24) * 128
                if qs < GRP:
                    spans = [(qs, GRP), (GRP, Q)] if qs < GRP else []
                else:
                    spans = [(qs, Q)]
                if qs == 0:
                    spans = [(0, GRP), (GRP, Q)]
                for a, b in spans:
                    nc.tensor.matmul(
                        s_ps[:, a:b],
                        kt_sb[:, jj * 128:(jj + 1) * 128],
                        qt_sb[h][:, a:b],
                        start=True, stop=True,
                    )
                nc.scalar.activation(
                    pt[:, qs:Q], s_ps[:, qs:Q],
                    mybir.ActivationFunctionType.Exp,
                    bias=shift_sb[:], scale=1.0,
                )
                # diagonal block masking: identical lower-triangular
                # [128,128] pattern for every jj >= 24, at cols [qs, qs+128)
                if jj >= 24:
                    nc.vector.tensor_mul(
                        pt[:, qs:qs + 128], pt[:, qs:qs + 128], tri_sb[:])
                # denominator accumulation (DVE, 2 bf16 accumulators)
                par = pos % 2
                dn = dn0 if par == 0 else dn1
                if par not in touched:
                    touched.add(par)
                    nc.vector.tensor_copy(dn[:], pt[:])
                else:
                    deng = (nc.gpsimd if pos == len(ORDER) - 1
                            else nc.vector)
                    deng.tensor_add(dn[:, qs:Q], dn[:, qs:Q], pt[:, qs:Q])

                pend.append((jj, pt))
                if len(pend) > 2:
                    emit_attn(pend.pop(0))
                if pos in (3, 9) and deferred_tail[0]:
                    deferred_tail[0].pop(0)()
                npop = 2 if pos < 8 else 1
                for _ in range(npop):
                    if fillers:
                        fillers.pop(0)()
            for ent in pend:
                emit_attn(ent)
            while fillers:
                fillers.pop(0)()
            # unnormalized attn out (frees a_ps for the next head)
            au_sb = smpool1.tile([128, Q], BF16, tag="atu", name="au_sb")
            nc.vector.tensor_copy(au_sb[:, 0:GRP], a_ps[:, 0:GRP])
            nc.scalar.copy(au_sb[:, GRP:Q], a_ps[:, GRP:Q])

            def make_tail(h, au_sb, dn0, dn1):
                rc_sb = smpool1.tile([2, GRP], BF16, tag="recip", name="rc_sb")

                def tail_ds():
                    ds_t = auxps.tile([128, GRP], F32, tag="aux", name="ds_t")
                    ds_ps = ds_t[0:2, :]
                    for idx, (sel, dn, hoff) in enumerate(
                        [(ones2a, dn0, 0), (ones2a, dn1, 0),
                         (ones2b, dn0, GRP), (ones2b, dn1, GRP)]
                    ):
                        nc.tensor.matmul(
                            ds_ps[:], sel[:], dn[:, hoff:hoff + GRP],
                            start=(idx == 0), stop=(idx == 3),
                        )
                    with nc.allow_low_precision(reason="1/denom bf16"):
                        nc.vector.reciprocal(rc_sb[:], ds_ps[:])

                def tail_bc():
                    for half in range(2):
                        hsl = slice(half * GRP, (half + 1) * GRP)
                        bc_ps = auxps.tile([128, GRP], F32, tag="aux",
                                           name="bc_ps")
                        nc.tensor.matmul(
                            bc_ps[:], sel_a if half == 0 else sel_b,
                            rc_sb[:], start=True, stop=True)
                        nc.vector.tensor_mul(
                            at_sb[h][:, hsl], au_sb[:, hsl], bc_ps[:])
                return [tail_ds, tail_bc]

            deferred_tail[0] = make_tail(h, au_sb, dn0, dn1)
        for fn in deferred_tail[0]:
            fn()

        # ---------------- stage 3: o_proj partial ----------------
        def finish_chunk(st_i, c, o_ps, h_start, opool):
            for hh in range(h_start, HPC):
                nc.tensor.matmul(
                    o_ps[:],
                    at_sb[hh][:, st_i * 128:(st_i + 1) * 128],
                    wo_sb[:, hh * HID + c * 512:hh * HID + (c + 1) * 512],
                    start=(hh == 0), stop=(hh == HPC - 1),
                )
            o_sb = opool.tile([128, 512], BF16, tag="osb", name="o_sb")
            if (st_i + c) % 2 == 0:
                nc.scalar.copy(o_sb[:], o_ps[:])
            else:
                nc.vector.tensor_copy(o_sb[:], o_ps[:])
            nc.sync.dma_start(
                outp[st_i * 128:(st_i + 1) * 128, c * 512:(c + 1) * 512],
                o_sb[:],
            )

        # complete the prefix chunks (st 0, c 0..N_PRE-1; h0/h1 already
        # accumulated during head 3), then release pools bottom-up and run
        # the remaining chunks with full PSUM
        for j in range(N_PRE):
            finish_chunk(0, j, pre_state[j], 2, ostpool)
        pre_stack.close()
        s2_stack.close()
        with (
            tc.tile_pool(name="ops", bufs=4, space="PSUM") as opps,
            tc.tile_pool(name="ostage2", bufs=4) as ostpool2,
        ):
            for st_i in range(8):
                for c in range(8):
                    if st_i == 0 and c < N_PRE:
                        continue
                    o_ps = opps.tile([128, 512], F32, tag="ops", name="o_ps")
                    finish_chunk(st_i, c, o_ps, 0, ostpool2)
        q2_stack.close()
        rot_stack.close()
    return nc


class TileCtx:
    """TileContext plus an ExitStack that closes before the context exits."""

    def __init__(self, nc):
        self.nc = nc

    def __enter__(self):
        self.tc = tile.TileContext(self.nc)
        self.tc.__enter__()
        self.st = ExitStack()
        return self.tc, self.st

    def __exit__(self, *exc):
        self.st.close()
        return self.tc.__exit__(*exc)


def _pack_ktiles(a, tile_rows=128):
    """[R, C] -> [128, (R//128)*C] with k-tile kt at cols [kt*C:(kt+1)*C]."""
    r, c = a.shape
    n = r // tile_rows
    return np.ascontiguousarray(
        a.reshape(n, tile_rows, c).transpose(1, 0, 2).reshape(tile_rows, n * c)
    )


def _rope_tables_ds(position_ids):
    """cos/sin in [d, s] layout: [128, Q] f64."""
    pos = np.asarray(position_ids).reshape(-1).astype(np.float64)      # [Q]
    inv_freq = 1.0 / (ROPE_THETA ** (np.arange(0, HD, 2, dtype=np.float64) / HD))
    ang_half = np.outer(inv_freq, pos)                                 # [64, Q]
    ang = np.concatenate([ang_half, ang_half], axis=0)                 # [128, Q]
    return np.cos(ang), np.sin(ang)


def kernel(hidden_states, attention_mask, position_ids, past_k, past_v,
           Wq, Wk, Wv, Wo):
    global LAST_RESULTS
    bf = ml_dtypes.bfloat16

    hs = np.asarray(hidden_states, np.float32).reshape(Q, HID)
    mask = np.asarray(attention_mask, np.float32).reshape(Q, KV)
    cos_d, sin_d = _rope_tables_ds(position_ids)

    scale = 1.0 / math.sqrt(HD)
    cosq_t = (cos_d * scale).astype(bf)
    sinq_t = (sin_d * scale).astype(bf)
    cosk_t = cos_d.astype(bf)
    sink_t = sin_d.astype(bf)

    # rotate-half permutation with sign: rot[d] = -x[d+64] (d<64); x[d-64]
    prot_np = np.zeros((128, 128), np.float32)
    for dd in range(64):
        prot_np[dd + 64, dd] = -1.0
        prot_np[dd, dd + 64] = 1.0
    prot_t = prot_np.astype(bf)

    # diagonal masks: [128 kv, 4 tiles * 512 q]: kv tile 24+m vs queries
    # 0..511 (identical pattern to kv tile 28+m vs queries 512..1023)
    mask_t = np.empty((128, 2048), np.float32)
    for m in range(4):
        kt = 24 + m
        blk = mask[0:512, kt * 128:(kt + 1) * 128].T
        mask_t[:, m * 512:(m + 1) * 512] = (blk == 0.0).astype(np.float32)
    mask_t = mask_t.astype(bf)
    # lower-triangular [128,128] diagonal-block mask (kv_sub <= q_sub),
    # derived from the input mask at (q 0:128, kv 3072:3200)
    tri_t = np.ascontiguousarray(
        (mask[0:128, PAST:PAST + 128] == 0.0).astype(np.float32).T).astype(bf)

    sel2_np = np.zeros((2, 256), np.float32)
    sel2_np[0, 0:128] = 1.0      # sel_a: broadcast rc row 0
    sel2_np[1, 128:256] = 1.0    # sel_b: broadcast rc row 1

    hst = _pack_ktiles(np.ascontiguousarray(hs.T)).astype(bf)
    hst = hst.reshape(128, N_HK, Q)

    nc = _build_program()
    in_maps = []
    for c in range(NCORES):
        ks = slice(c * HD, (c + 1) * HD)
        wq_heads = []
        for h in range(HPC):
            rows = slice((HPC * c + h) * HD, (HPC * c + h + 1) * HD)
            wq_heads.append(
                _pack_ktiles(np.ascontiguousarray(Wq[rows, :].T))
                .reshape(128, N_HK, 128)
            )
        wq_c = np.ascontiguousarray(
            np.stack(wq_heads, axis=1)).astype(bf)                 # [128,4,32,128]
        wk_c = np.ascontiguousarray(Wk[ks, :].T)                   # [4096, 128]
        wv_c = np.ascontiguousarray(Wv[ks, :].T)
        wkv_c = _pack_ktiles(
            np.concatenate([wk_c, wv_c], axis=1)
        ).astype(bf).reshape(128, N_HK, 256)
        pkt = np.ascontiguousarray(past_k[0, c].T).astype(bf)      # [128, 3072]
        pv = _pack_ktiles(np.ascontiguousarray(past_v[0, c])).astype(bf)
        qs = slice(c * HPC * HD, (c + 1) * HPC * HD)
        wo_c = _pack_ktiles(
            np.ascontiguousarray(Wo[:, qs].T)).astype(bf)          # [128, 4*4096]
        in_maps.append({
            "hst": hst, "wqt": wq_c, "wkvt": wkv_c, "pastkt": pkt,
            "pastv": pv, "maskt": mask_t, "cosq": cosq_t, "sinq": sinq_t,
            "cosk": cosk_t, "sink": sink_t, "prot": prot_t, "trit": tri_t,
            "sel2": sel2_np.astype(bf),
            "wot": wo_c,
        })

    res = run_bass_kernel_spmd(nc, in_maps, list(range(NCORES)))
    LAST_RESULTS = res
    out = np.zeros((Q, HID), np.float32)
    for c in range(NCORES):
        out += np.asarray(res.results[c]["outp"], dtype=np.float32)
    return out.reshape(B, Q, HID)
